# revision 1
# baseline (speedup 1.0000x reference)
"""Trainium2 Bass kernel for the gated dual-softmax attention problem.

Shapes (hardcoded): x [4,1024,256], pos [4,1024,16], H=8 heads, dh=32.

Math notes (exact reformulations of the reference):
  * pos_logits[b,h,i,j] = (p[b,i]-p[b,j])@Wh[:,h] + bh[h].  Under softmax
    over j the i-dependent terms are constants, so
    pos_attn[b,h,i,j] = softmax_j(-p[b,j]@Wh[:,h]) =: w[b,h,j]  (no i dep).
    Its output contribution is rank-1: g_h*(w_h@v_h) for every query row,
    folded into an effective bias bo_eff added after the Wo projection.
  * Both softmaxes sum to 1, so the renormalization is an exact no-op.
  * (1-g_h) scaling of the qk branch is folded into Wo rows (host side);
    the pos-branch compensates with a g/(1-g) factor.

Sharding: 8 cores = 4 batches x 2 query-halves; no cross-core math.

Per-core pipeline (engine-balanced against the TimelineSim cost model):
  * scoresT[j,i] per (head-group mc, key-chunk kc, query-half qh) via
    4 row-tiled matmuls (lhsT = kT 32x128 slice, rhs = qT) into a
    [128,4,256] psum tile (2 banks, 2-deep rotation).
  * exp runs on ACT (activation Exp -> bf16) or DVE (Schraudolph:
    int16(x*a+b) bit-cast as bf16, one fused tensor_scalar) per a static
    assignment table, so the ~27us of exp work is split across engines.
  * PV is flipped: out[i, 0:33] += et[j,i].T @ v_aug[j, 0:33], i.e. the
    bf16 et tile is the stationary operand and the 33-wide bf16 v_aug
    (col 32 = ones) is the moving operand -> 33-cycle matmuls, full-width
    psum accumulation in a single [128, 4,2,4,33] psum tile.
  * epilogue: reciprocal of col 32 (per-partition) + broadcast multiply,
    both free-axis ops on DVE; transpose o via PE; project with bf16 Wo;
    bo_eff added as an extra rank-1 matmul into the same psum group.
"""

import sys

if "/opt/trn_rl_repo" not in sys.path:
    sys.path.insert(0, "/opt/trn_rl_repo")

import numpy as np

B, N, D, H, DH, DP, PD = 4, 1024, 256, 8, 32, 32, 16
NQ = N // 2          # query rows per core
NCORES = 8
INV_C = 1.0 / np.sqrt(DH)
LOG2E = 1.4426950408889634
# Schraudolph constants for bf16-bitcast exp: i16 = s*(INV_C*128*log2e) + SB
SA = INV_C * 128.0 * LOG2E
SB = 16256.0 - 5.0

# Which (t, qh) score subtiles get their exp on DVE (rest on ACT).
# t = mc*8 + kc indexes the 16 (head-group, key-chunk) steps.
DVE_EXP = {(3, 1), (4, 1), (6, 1), (7, 1), (9, 1), (10, 1), (11, 1),
           (12, 1), (13, 1), (14, 1)}

import os
ABL_NO_DVE_EXP = os.environ.get("ABL_NO_DVE_EXP", "0") == "1"
ABL_NO_GPSIMD_DMA = os.environ.get("ABL_NO_GPSIMD_DMA", "0") == "1"
ABL_NO_GPSIMD_OPS = os.environ.get("ABL_NO_GPSIMD_OPS", "0") == "1"
ABL_NO_POS = os.environ.get("ABL_NO_POS", "0") == "1"
ABL_STAGE = int(os.environ.get("ABL_STAGE", "5"))
ABL_NO_LEFT = os.environ.get("ABL_NO_LEFT", "0") == "1"
ABL_SIMPLE_SCORES = os.environ.get("ABL_SIMPLE_SCORES", "0") == "1"
ABL_NT = int(os.environ.get("ABL_NT", "16"))

_nc_cache = {}


def _build_nc():
    from contextlib import ExitStack

    import concourse.bass as bass
    import concourse.tile as tile
    from concourse import bacc, mybir

    f32 = mybir.dt.float32
    f32r = mybir.dt.float32r
    bf16 = mybir.dt.bfloat16
    i16 = mybir.dt.int16
    AL = mybir.AluOpType
    EXP = mybir.ActivationFunctionType.Exp

    nc = bacc.Bacc("TRN2", target_bir_lowering=False, debug=False,
                   num_devices=NCORES)

    din = {}
    for name, shape in [
        ("xq", [128, 2, NQ]), ("xkv", [128, 2, N]),
        ("qkw", [128, 2, 2 * D]), ("wvo", [128, 2, 2 * D]),
    ]:
        din[name] = nc.dram_tensor(name, shape, bf16,
                                   kind="ExternalInput").ap()
    for name, shape in [("posT", [PD, N]), ("blob", [32, 640])]:
        din[name] = nc.dram_tensor(name, shape, f32, kind="ExternalInput").ap()
    dout = nc.dram_tensor("out", [NQ, D], f32, kind="ExternalOutput").ap()
    dout_r = dout.rearrange("(c p) d -> p c d", c=4)

    with tile.TileContext(nc) as tc, ExitStack() as ctx:
        P = ctx.enter_context(tc.tile_pool(name="P", bufs=1))
        ps_sc = ctx.enter_context(tc.tile_pool(name="ps_sc", bufs=3,
                                               space="PSUM"))
        ps_pv = ctx.enter_context(tc.tile_pool(name="ps_pv", bufs=1,
                                               space="PSUM"))

        def sc_tile(name, path=None):
            return ps_sc.tile([128, 2, NQ], f32, tag="sc", name=name)

        # ---- input DMAs, spread across issue queues ----
        xkv = P.tile([128, 2, N], bf16, tag="xkv")
        nc.sync.dma_start(out=xkv, in_=din["xkv"])
        qkw = P.tile([128, 2, 2 * D], bf16, tag="qkw")
        nc.scalar.dma_start(out=qkw, in_=din["qkw"])
        wq = qkw[:, :, 0:D]
        wk = qkw[:, :, D:2 * D]
        xq = P.tile([128, 2, NQ], bf16, tag="xq")
        nc.sync.dma_start(out=xq, in_=din["xq"])
        # exp table preload (first ACT activation in stream)
        dummy = P.tile([1, 8], f32, tag="dummy")
        (nc.vector if ABL_NO_GPSIMD_OPS else nc.gpsimd).memset(dummy, 0.0)
        dummy2 = P.tile([1, 8], bf16, tag="dummy2")
        nc.scalar.activation(out=dummy2, in_=dummy, func=EXP)  # table load

        wvo = P.tile([128, 2, 2 * D], bf16, tag="wvo")
        nc.scalar.dma_start(out=wvo, in_=din["wvo"])
        wv = wvo[:, :, 0:D]
        wo_bf = wvo[:, :, D:2 * D]
        blob = P.tile([32, 640], f32r, tag="blob")
        posT = P.tile([PD, N], f32r, tag="posT")
        if ABL_NO_GPSIMD_DMA:
            blob_f = P.tile([32, 640], f32, tag="blob_f")
            nc.sync.dma_start(out=blob_f, in_=din["blob"])
            nc.vector.tensor_copy(out=blob, in_=blob_f)
            posT_f = P.tile([PD, N], f32, tag="posT_f")
            nc.sync.dma_start(out=posT_f, in_=din["posT"])
            nc.vector.tensor_copy(out=posT, in_=posT_f)
        else:
            nc.gpsimd.dma_start(out=blob, in_=din["blob"])
            nc.gpsimd.dma_start(out=posT, in_=din["posT"])

        wp1 = blob[0:PD, 0:PD]
        wp2 = blob[0:PD, PD:PD + DP]
        wh = blob[0:DP, 48:56]
        bp1 = blob[0:PD, 56:57].bitcast(f32)
        gdm = blob[0:H, 57:58].bitcast(f32)
        bo_row = blob[0:1, 320:576].bitcast(f32)

        # PE warmup: dep-free matmuls into a scratch psum region ramp the
        # tensor engine to its fast p-state while input DMAs are in flight.
        warm_src = P.tile([128, NQ], bf16, tag="warm_src")
        nc.vector.memset(warm_src, 0.0)
        wtl = sc_tile("warm")
        for i in range(12):
            nc.tensor.matmul(wtl[:, 0, 0:256], lhsT=warm_src[:, 0:128],
                             rhs=warm_src[:, 0:256], start=True, stop=True)

        # persistent sbuf state
        ones_bf = P.tile([1, 128], bf16, tag="ones_bf")
        (nc.vector if ABL_NO_GPSIMD_OPS else nc.gpsimd).memset(ones_bf, 1.0)
        ident = P.tile([128, 128], f32, tag="ident")
        from concourse.masks import make_identity
        make_identity(nc, ident[:])
        ident_bf = P.tile([128, 128], bf16, tag="ident_bf")
        (nc.vector if ABL_NO_GPSIMD_OPS else nc.gpsimd).tensor_copy(
            out=ident_bf, in_=ident)

        v_aug = P.tile([128, 8, H, DH + 1], bf16, tag="v_aug")
        (nc.vector if ABL_NO_GPSIMD_OPS else nc.gpsimd).memset(
            v_aug[:, :, :, DH], 1.0)

        qT = [P.tile([128, NQ], f32r, tag=f"qT{mc}", name=f"qT{mc}")
              for mc in range(2)]
        kT = [P.tile([128, N], f32r, tag=f"kT{mc}", name=f"kT{mc}")
              for mc in range(2)]
        et = [P.tile([128, 2, NQ], bf16, tag=f"et{s}", name=f"et{s}")
              for s in range(32)]
        onorm = [P.tile([128, D], bf16, tag=f"on{qt}", name=f"on{qt}")
                 for qt in range(4)]
        oT = P.tile([128, 2, NQ], bf16, tag="oT")
        out_sb = P.tile([128, 4, D], f32, tag="out_sb")
        h1 = P.tile([PD, N], f32r, tag="h1")
        p_sb = P.tile([DP, N], f32r, tag="p_sb")
        ep = P.tile([H, N], f32, tag="ep")
        eps_bf = P.tile([H, N], bf16, tag="eps_bf")
        epT = P.tile([128, 8, H], bf16, tag="epT")
        gr = P.tile([H, 1], f32, tag="gr")
        rp = P.tile([H, 1], f32, tag="rp")
        spsum = P.tile([H, 1], f32, tag="spsum")
        u_bf = P.tile([128, 2, 1], bf16, tag="u_bf")
        bo_eff = P.tile([1, D], bf16, tag="bo_eff")
        recip = [P.tile([128, 2, 4], f32, tag=f"rc{g}", name=f"rc{g}")
                 for g in range(4)]

        # PV accumulators: per (qh) tile [128, ic, ht, 33], bank-padded;
        # bufs=2 rotates between the two mc phases.
        def pv_tile(qh, mc):
            return ps_pv.tile([128, 2, 4, DH + 1], f32, tag=f"pv{qh}",
                              padded_shape=[128, 2, 4, 64],
                              name=f"pv{qh}_{mc}")
        pv_cur = {}

        # ---- projections for mc=0 (critical path to first scores) ----
        def proj_q(mc):
            t = sc_tile(f"qp{mc}")
            p = t[:].rearrange("p a b -> p (a b)")[:, 0:NQ]
            for kc in range(2):
                nc.tensor.matmul(p, lhsT=wq[:, kc, mc * 128:(mc + 1) * 128],
                                 rhs=xq[:, kc, :], start=(kc == 0),
                                 stop=(kc == 1))
            nc.vector.tensor_copy(out=qT[mc], in_=p)

        def proj_k(mc):
            t = sc_tile(f"kp{mc}")
            p = t[:].rearrange("p a b -> p (a b)")
            for nn in range(2):
                for kc in range(2):
                    nc.tensor.matmul(
                        p[:, nn * NQ:(nn + 1) * NQ],
                        lhsT=wk[:, kc, mc * 128:(mc + 1) * 128],
                        rhs=xkv[:, kc, nn * NQ:(nn + 1) * NQ],
                        start=(kc == 0), stop=(kc == 1))
                nc.vector.tensor_copy(
                    out=kT[mc][:, nn * NQ:(nn + 1) * NQ],
                    in_=p[:, nn * NQ:(nn + 1) * NQ])

        def proj_v(jc):
            t = sc_tile(f"vp{jc}")
            p = t[:].rearrange("p a b -> p (a b)")[:, 0:D]
            for kc in range(2):
                nc.tensor.matmul(p, lhsT=xkv[:, kc, jc * 128:(jc + 1) * 128],
                                 rhs=wv[:, kc, :], start=(kc == 0),
                                 stop=(kc == 1))
            nc.vector.tensor_copy(
                out=v_aug[:, jc, :, 0:DH],
                in_=p.rearrange("p (h d) -> p h d", h=H))

        if ABL_STAGE >= 1:
            proj_k(0)
            proj_q(0)

        # ---- pos branch pieces (emitted as leftovers inside the loop) ----
        def pos_h1():
            t = sc_tile("posh")
            p = t[:].rearrange("p a b -> p (a b)")[0:PD, :]
            for nn in range(2):
                nc.tensor.matmul(p[:, nn * NQ:(nn + 1) * NQ], lhsT=wp1,
                                 rhs=posT[:, nn * NQ:(nn + 1) * NQ],
                                 start=True, stop=True)
            nc.vector.tensor_scalar(out=h1, in0=p, scalar1=bp1, scalar2=0.0,
                                    op0=AL.add, op1=AL.max)

        def pos_p():
            t = sc_tile("posp")
            p = t[:].rearrange("p a b -> p (a b)")[0:DP, :]
            for nn in range(2):
                nc.tensor.matmul(p[:, nn * NQ:(nn + 1) * NQ], lhsT=wp2,
                                 rhs=h1[:, nn * NQ:(nn + 1) * NQ],
                                 start=True, stop=True)
            nc.vector.tensor_copy(out=p_sb, in_=p)

        def pos_sp():
            t = sc_tile("possp")
            p = t[:].rearrange("p a b -> p (a b)")[0:H, :]
            for nn in range(2):
                nc.tensor.matmul(p[:, nn * NQ:(nn + 1) * NQ], lhsT=wh,
                                 rhs=p_sb[:, nn * NQ:(nn + 1) * NQ],
                                 start=True, stop=True)
            # w ~ softmax_j(-sp): exp(-sp) (range is modest, no max-sub)
            nc.scalar.activation(out=ep, in_=p, func=EXP, scale=-1.0)
            nc.vector.tensor_reduce(out=spsum, in_=ep,
                                    axis=mybir.AxisListType.X, op=AL.add)
            nc.vector.reciprocal(out=rp, in_=spsum)
            nc.vector.tensor_tensor(out=gr, in0=gdm, in1=rp, op=AL.mult)
            (nc.vector if ABL_NO_GPSIMD_OPS else nc.gpsimd).tensor_scalar(
                out=eps_bf, in0=ep, scalar1=gr,
                scalar2=0.0, op0=AL.mult, op1=AL.add)

        def pos_epT():
            # two transposes per sc tile (one per 2KB bank), 4 tiles total
            for tt in range(4):
                t = sc_tile(f"posep{tt}")
                p = t[:].rearrange("p a b -> p (a b)").bitcast(bf16)
                for half in range(2):
                    jc = tt * 2 + half
                    nc.tensor.transpose(
                        p[:, half * 1024:half * 1024 + H],
                        eps_bf[:, jc * 128:(jc + 1) * 128],
                        ident_bf[0:H, 0:H])
                nc.vector.tensor_copy(
                    out=epT[:, tt * 2:tt * 2 + 2, :],
                    in_=p.rearrange("p (a b) -> p a b", a=2)[:, :, 0:H])

        def pos_u():
            t = sc_tile("posu")
            p = t[:].rearrange("p a b -> p (a b)")
            for h in range(H):
                hp = (h % 4) * DH
                co = (h // 4) * 512          # separate 2KB bank per u column
                for jc in range(8):
                    nc.tensor.matmul(
                        p[hp:hp + DH, co:co + 1],
                        lhsT=v_aug[:, jc, h, 0:DH],
                        rhs=epT[:, jc, h:h + 1],
                        start=(jc == 0), stop=(jc == 7),
                        tile_position=(0, hp))
            nc.vector.tensor_copy(
                out=u_bf,
                in_=p.rearrange("p (a b) -> p a b", a=2)[:, :, 0:1])

        def pos_bo():
            t = sc_tile("posbo")
            p = t[:].rearrange("p a b -> p (a b)")[0:1, 0:D]
            for mc in range(2):
                nc.tensor.matmul(p, lhsT=u_bf[:, mc, :],
                                 rhs=wo_bf[:, mc, :], start=(mc == 0),
                                 stop=(mc == 1))
            nc.vector.tensor_tensor(out=bo_eff, in0=p, in1=bo_row, op=AL.add)

        def sc_pad():
            t = sc_tile("pad")
            nc.tensor.matmul(t[:, 0, 0:64], lhsT=warm_src[:, 0:128],
                             rhs=warm_src[:, 0:64], start=True, stop=True)

        # epilogue for pv group g=(mc,qh): normalize and write onorm columns
        def epilogue(g):
            mc, qh = g // 2, g % 2
            pvt = pv_cur[(mc, qh)]
            nc.vector.reciprocal(out=recip[g], in_=pvt[:, :, :, DH:DH + 1])
            for ic in range(2):
                qt = qh * 2 + ic
                dst = onorm[qt][:, mc * 128:(mc + 1) * 128] \
                    .rearrange("p (h d) -> p h d", h=4)
                nc.vector.tensor_tensor(
                    out=dst, in0=pvt[:, ic, :, 0:DH],
                    in1=recip[g][:, ic, :].unsqueeze(2)
                        .broadcast_to([128, 4, DH]),
                    op=AL.mult)

        # transpose onorm[qt] -> oT[:, :, qt-slice]; then project+store
        def out_tr(qt):
            t = sc_tile(f"tr{qt}")
            p = t[:].rearrange("p a b -> p (a b)").bitcast(bf16)
            for mcd in range(2):
                nc.tensor.transpose(p[:, mcd * 1024:mcd * 1024 + 128],
                                    onorm[qt][:, mcd * 128:(mcd + 1) * 128],
                                    ident_bf)
            nc.scalar.copy(
                out=oT[:, :, qt * 128:(qt + 1) * 128],
                in_=p.rearrange("p (a b) -> p a b", a=2)[:, :, 0:128])

        def out_proj(qt):
            t2 = sc_tile(f"op{qt}")
            po = t2[:].rearrange("p a b -> p (a b)")[:, 0:D]
            for mcd in range(2):
                nc.tensor.matmul(po, lhsT=oT[:, mcd, qt * 128:(qt + 1) * 128],
                                 rhs=wo_bf[:, mcd, :], start=(mcd == 0),
                                 stop=False)
            nc.tensor.matmul(po, lhsT=ones_bf, rhs=bo_eff, start=False,
                             stop=True)
            if qt % 2 == 0:
                nc.scalar.copy(out=out_sb[:, qt, :], in_=po)
            else:
                nc.vector.tensor_copy(out=out_sb[:, qt, :], in_=po)
            if qt % 2 == 0:
                nc.sync.dma_start(out=dout_r[:, qt, :], in_=out_sb[:, qt, :])
            else:
                nc.scalar.dma_start(out=dout_r[:, qt, :],
                                    in_=out_sb[:, qt, :])

        # ---- main software-pipelined attention loop ----
        def scores(t, p, tl):
            mc = t // 8
            kc = t % 8
            for h2 in range(2):
                ht = 2 * p + h2
                nc.tensor.matmul(
                    tl[:, h2, :],
                    lhsT=kT[mc][ht * DH:(ht + 1) * DH,
                                kc * 128:(kc + 1) * 128],
                    rhs=qT[mc][ht * DH:(ht + 1) * DH, :],
                    start=True, stop=True,
                    tile_position=(ht * DH, 0))

        def exp_sub(t, p, tl):
            s = t * 2 + p
            if (t, p) in DVE_EXP and not ABL_NO_DVE_EXP:
                nc.vector.tensor_scalar(out=et[s][:].bitcast(i16), in0=tl,
                                        scalar1=SA, scalar2=SB,
                                        op0=AL.mult, op1=AL.add)
            else:
                nc.scalar.activation(out=et[s], in_=tl, func=EXP, scale=INV_C)

        def pv_mms(t):
            mc = t // 8
            kc = t % 8
            if kc == 0:
                for qh in range(2):
                    pv_cur[(mc, qh)] = pv_tile(qh, mc)
            for qh in range(2):
                pvt = pv_cur[(mc, qh)]
                for p in range(2):
                    s = t * 2 + p
                    for h2 in range(2):
                        ht = 2 * p + h2
                        for icq in range(2):
                            ic = qh * 2 + icq
                            nc.tensor.matmul(
                                pvt[:, icq, ht, :],
                                lhsT=et[s][:, h2, ic * 128:(ic + 1) * 128],
                                rhs=v_aug[:, kc, mc * 4 + ht, :],
                                start=(kc == 0 and p == 0 and h2 == 0
                                       and icq == 0),
                                stop=(kc == 7 and p == 1 and h2 == 1
                                      and icq == 1))

        if ABL_NO_POS:
            # bo_eff = bo only
            nc.vector.tensor_copy(out=bo_eff, in_=bo_row)
        if ABL_NO_LEFT:
            nc.vector.memset(qT[1][:].bitcast(f32), 0.0)
            nc.vector.memset(kT[1][:].bitcast(f32), 0.0)
            nc.vector.memset(v_aug[:], 0.0)
        leftovers = {} if ABL_NO_LEFT else {
            0: [lambda: proj_v(0), lambda: proj_v(1)],
            1: [lambda: proj_v(2), lambda: proj_v(3)],
            2: [lambda: proj_v(4), lambda: proj_v(5)],
            3: [lambda: proj_v(6), lambda: proj_v(7)],
            4: [lambda: proj_q(1)],
            5: [lambda: proj_k(1)],
            6: [] if ABL_NO_POS else [pos_h1],
            7: [] if ABL_NO_POS else [pos_p],
            8: [] if ABL_NO_POS else [pos_sp],
            9: ([] if ABL_NO_POS else [pos_epT]) +
               ([lambda: epilogue(0)] if ABL_STAGE >= 4 else []),
            10: ([] if ABL_NO_POS else [pos_u]) +
                ([lambda: epilogue(1)] if ABL_STAGE >= 4 else []),
            11: [] if ABL_NO_POS else [pos_bo],
        }

        for t in range(min(ABL_NT, 16) if ABL_STAGE >= 1 else 0):
            for p in range(2):
                tl = sc_tile(f"s{t}_{p}", path=p)
                scores(t, p, tl)
                if ABL_STAGE >= 2:
                    exp_sub(t, p, tl)
            for fn in leftovers.get(t, []):
                fn()
            if t > 0 and ABL_STAGE >= 3:
                pv_mms(t - 1)
        if ABL_STAGE >= 3:
            pv_mms(15)
        if ABL_STAGE >= 4:
            epilogue(2)
            epilogue(3)
        if ABL_STAGE >= 5:
            for qt in range(4):
                out_tr(qt)
            for qt in range(4):
                out_proj(qt)
        else:
            nc.vector.memset(out_sb, 0.0)
            for qt in range(4):
                nc.scalar.dma_start(out=dout_r[:, qt, :], in_=out_sb[:, qt, :])

    nc.compile()
    return nc


def _get_nc():
    if "nc" not in _nc_cache:
        _nc_cache["nc"] = _build_nc()
    return _nc_cache["nc"]


def _host_prep(inputs):
    x = np.ascontiguousarray(np.asarray(inputs["x"], dtype=np.float32))
    pos = np.ascontiguousarray(np.asarray(inputs["pos"], dtype=np.float32))
    Wq = np.asarray(inputs["Wq"], np.float32)
    Wk = np.asarray(inputs["Wk"], np.float32)
    Wv = np.asarray(inputs["Wv"], np.float32)
    Wo = np.asarray(inputs["Wo"], np.float32)
    bo = np.asarray(inputs["bo"], np.float32).reshape(1, D)
    gate = np.asarray(inputs["gate"], np.float32)
    g = (1.0 / (1.0 + np.exp(-gate.astype(np.float64))))
    omg = (1.0 - g)                          # (1-g), float64
    Wo_s = (Wo.astype(np.float64) *
            np.repeat(omg, DH)[:, None]).astype(np.float32)
    gdm = (g / omg).astype(np.float32).reshape(H, 1)

    import ml_dtypes

    def pack2(w):
        return np.ascontiguousarray(
            np.stack([w[0:128], w[128:256]], axis=1)
            .astype(ml_dtypes.bfloat16))

    blob = np.zeros((32, 640), np.float32)
    blob[0:PD, 0:PD] = np.asarray(inputs["Wp1"], np.float32)
    blob[0:PD, PD:PD + DP] = np.asarray(inputs["Wp2"], np.float32)
    blob[0:DP, 48:56] = np.asarray(inputs["Wh"], np.float32)
    blob[0:PD, 56:57] = np.asarray(inputs["bp1"], np.float32).reshape(PD, 1)
    blob[0:H, 57:58] = gdm
    blob[0:1, 320:576] = bo

    per_core = []
    for core in range(NCORES):
        b, half = divmod(core, 2)
        q0 = half * NQ
        xT = np.ascontiguousarray(x[b].T)           # [256, 1024]
        per_core.append({
            "xq": pack2(np.ascontiguousarray(xT[:, q0:q0 + NQ])),
            "xkv": pack2(xT),
            "posT": np.ascontiguousarray(pos[b].T),
            "qkw": pack2(np.concatenate([Wq, Wk], axis=1)),
            "wvo": pack2(np.concatenate([Wv, Wo_s], axis=1)),
            "blob": blob,
        })
    return per_core


def kernel(**inputs):
    from concourse.bass_utils import run_bass_kernel_spmd

    nc = _get_nc()
    in_maps = _host_prep(inputs)
    res = run_bass_kernel_spmd(nc, in_maps, core_ids=list(range(NCORES)))
    out = np.empty((B, N, D), np.float32)
    for core in range(NCORES):
        b, half = divmod(core, 2)
        out[b, half * NQ:(half + 1) * NQ, :] = res.results[core]["out"]
    return out



# revision 55
# speedup vs baseline: 1.1437x; 1.1437x over previous
"""Trainium2 Bass kernel for the gated dual-softmax attention problem.

Shapes (hardcoded): x [4,1024,256], pos [4,1024,16], H=8 heads, dh=32.

Math notes (exact reformulations of the reference):
  * pos_logits[b,h,i,j] = (p[b,i]-p[b,j])@Wh[:,h] + bh[h].  Under softmax
    over j the i-dependent terms are constants, so
    pos_attn[b,h,i,j] = softmax_j(-p[b,j]@Wh[:,h]) =: w[b,h,j]  (no i dep).
    Its output contribution is rank-1: g_h*(w_h@v_h) for every query row,
    folded into an effective bias bo_eff added after the Wo projection.
  * sp = p@Wh = (relu(pos@Wp1+bp1)@Wp2+bp2)@Wh = relu(...)@(Wp2@Wh) + cb
    with w2h = Wp2@Wh and cb = bp2@Wh + bh folded on host, so the pos-MLP
    second layer disappears from the device program.
  * Both softmaxes sum to 1, so the renormalization is an exact no-op.
  * (1-g_h) scaling of the qk branch is folded into Wo rows (host side);
    the pos-branch compensates with a g/(1-g) factor.

Sharding: 8 cores = 4 batches x 2 query-halves; no cross-core math.

Per-core pipeline:
  * q/k projected in bf16; qT/kT stored fp8e4m3 in [128, 2, N] tiles whose
    dim-1 slot 1 is a zero strip, so the QK^T matmuls run in DoubleRow
    perf mode (0.5 cycles/row: 107ns per head-chunk instead of 213).
  * exp runs on ACT (activation Exp -> bf16) or DVE (Schraudolph:
    int16(x*a+b) bit-cast as bf16, one fused tensor_scalar) per a static
    assignment table, splitting ~34us of exp work across both engines.
  * PV is flipped: out[i, 0:33] += et[j,i].T @ v_aug[j, 0:33] with the
    bf16 et tile stationary -> 33-cycle matmuls, accumulated per
    (mc, qh) in a single-bank psum tile.
  * epilogue: reciprocal of col 32 + one fused broadcast multiply per
    group (DVE); transpose o via PE (mc=0 halves mid-loop, mc=1 at the
    tail); project with bf16 Wo; bo_eff added as a rank-1 matmul.
"""

import os
import sys

if "/opt/trn_rl_repo" not in sys.path:
    sys.path.insert(0, "/opt/trn_rl_repo")

import numpy as np

B, N, D, H, DH, DP, PD = 4, 1024, 256, 8, 32, 32, 16
NQ = N // 2          # query rows per core
NCORES = 8
INV_C = 1.0 / np.sqrt(DH)
LOG2E = 1.4426950408889634
# Schraudolph constants for bf16-bitcast exp: i16 = s*(INV_C*128*log2e) + SB
SA = INV_C * 128.0 * LOG2E
SB = 16256.0 - 5.0

# fp8e4m3 DoubleRow scores measured 3.8e-2 rel err on HW (e4m3
# quantization of q/k alone costs ~5% on the qk branch) -- keep bf16.
ABL_FP8 = os.environ.get("ABL_FP8", "0") == "1"
ABL_NT = int(os.environ.get("ABL_NT", "16"))
# EXP_SPLIT > 0: every main-loop exp tile is split at column EXP_SPLIT
# between ACT (low half) and DVE (high half), releasing the psum score
# tile sooner (the 3-deep psum ring paces the whole loop).
EXP_SPLIT = int(os.environ.get("EXP_SPLIT", "0"))
W1 = int(os.environ.get("W1", "20"))
W2 = int(os.environ.get("W2", "4"))
LSCHED = int(os.environ.get("LSCHED", "1"))
TAILV = int(os.environ.get("TAILV", "0"))

# (t, p) pairs whose exp runs on DVE (rest on ACT); used when EXP_SPLIT=0.
DVE_EXP = {(t, 1) for t in (2, 3, 4, 5, 6, 7, 8, 9, 11, 12, 13, 14, 15)}

_nc_cache = {}


def _build_nc():
    from contextlib import ExitStack

    import concourse.bass as bass
    import concourse.tile as tile
    from concourse import bacc, mybir

    f32 = mybir.dt.float32
    f32r = mybir.dt.float32r
    bf16 = mybir.dt.bfloat16
    fp8 = mybir.dt.float8e4
    i16 = mybir.dt.int16
    AL = mybir.AluOpType
    EXP = mybir.ActivationFunctionType.Exp
    DR = mybir.MatmulPerfMode.DoubleRow if ABL_FP8 else None
    qk_dt = fp8 if ABL_FP8 else bf16

    nc = bacc.Bacc("TRN2", target_bir_lowering=False, debug=False,
                   num_devices=NCORES)

    din = {}
    for name, shape in [
        ("xq", [128, 2, NQ]), ("xkv", [128, 2, N]),
        ("qkw", [128, 2, 2 * D]), ("wvo", [128, 2, 2 * D]),
    ]:
        din[name] = nc.dram_tensor(name, shape, bf16,
                                   kind="ExternalInput").ap()
    for name, shape in [("posT", [32, N]), ("blob", [32, 640])]:
        din[name] = nc.dram_tensor(name, shape, f32, kind="ExternalInput").ap()
    dout = nc.dram_tensor("out", [NQ, D], f32, kind="ExternalOutput").ap()
    dout_r = dout.rearrange("(c p) d -> p c d", c=4)

    with tile.TileContext(nc) as tc, ExitStack() as ctx:
        P = ctx.enter_context(tc.tile_pool(name="P", bufs=1))
        ps_sc = ctx.enter_context(tc.tile_pool(name="ps_sc", bufs=3,
                                               space="PSUM"))
        ps_pv = ctx.enter_context(tc.tile_pool(name="ps_pv", bufs=1,
                                               space="PSUM"))

        def sc_tile(name):
            return ps_sc.tile([128, 2, NQ], f32, tag="sc", name=name)

        # ---- persistent sbuf state ----
        warm_src = P.tile([128, 128], bf16, tag="warm_src")
        nc.gpsimd.memset(warm_src, 0.0)
        dummy = P.tile([1, 8], f32, tag="dummy")
        nc.gpsimd.memset(dummy, 0.0)

        qT8 = [P.tile([128, 2, NQ], qk_dt, tag=f"qT8{mc}", name=f"qT8{mc}")
               for mc in range(2)]
        kT8 = [P.tile([128, 2, N], qk_dt, tag=f"kT8{mc}", name=f"kT8{mc}")
               for mc in range(2)]
        v_aug = P.tile([128, 8, H, DH + 1], bf16, tag="v_aug")
        if ABL_FP8:
            nc.gpsimd.memset(qT8[0][:, 1, :], 0.0)
            nc.gpsimd.memset(kT8[0][:, 1, :], 0.0)
        nc.gpsimd.memset(v_aug[:, :, :, DH], 1.0)
        ones_bf = P.tile([1, 128], bf16, tag="ones_bf")
        nc.gpsimd.memset(ones_bf, 1.0)

        # ---- input DMAs ----
        # critical path (q/k) on sync-queue HWDGE, bulk weights on gpsimd
        # SWDGE; slices ordered so the first scores can start earliest.
        xq = P.tile([128, 2, NQ], bf16, tag="xq")
        qkw = P.tile([128, 2, 2 * D], bf16, tag="qkw")
        xkv = P.tile([128, 2, N], bf16, tag="xkv")
        wq = qkw[:, :, 0:D]
        wk = qkw[:, :, D:2 * D]
        nc.sync.dma_start(out=qkw, in_=din["qkw"])
        nc.sync.dma_start(out=xq, in_=din["xq"])
        nc.sync.dma_start(out=xkv[:, :, 0:NQ], in_=din["xkv"][:, :, 0:NQ])
        nc.sync.dma_start(out=xkv[:, :, NQ:N], in_=din["xkv"][:, :, NQ:N])

        wvo = P.tile([128, 2, 2 * D], bf16, tag="wvo")
        nc.gpsimd.dma_start(out=wvo, in_=din["wvo"])
        wv = wvo[:, :, 0:D]
        wo_bf = wvo[:, :, D:2 * D]
        blob = P.tile([32, 640], f32r, tag="blob")
        posT = P.tile([32, N], f32r, tag="posT")
        nc.gpsimd.dma_start(out=posT, in_=din["posT"])
        nc.gpsimd.dma_start(out=blob, in_=din["blob"])
        if ABL_FP8:
            nc.gpsimd.memset(qT8[1][:, 1, :], 0.0)
            nc.gpsimd.memset(kT8[1][:, 1, :], 0.0)

        wp1 = blob[0:32, 0:32]
        w2h = blob[0:32, 48:56]
        bp1r = blob[0:32, 56:57].bitcast(f32)
        gdm = blob[0:H, 57:58].bitcast(f32)
        ncb = blob[0:H, 58:59].bitcast(f32)
        bo_row = blob[0:1, 320:576].bitcast(f32)

        # exp table preload (first ACT activation in stream)
        dummy2 = P.tile([1, 8], bf16, tag="dummy2")
        nc.scalar.activation(out=dummy2, in_=dummy, func=EXP)  # table load

        # PE warmup: dep-free matmuls ramp the tensor engine to its fast
        # p-state while the input DMAs are in flight; fine-grained so the
        # first projection isn't delayed behind a long warmup matmul.
        # Sized to bridge until the q/k inputs land -- any PE idle gap
        # resets the p-state ramp and doubles early-matmul cost.
        wtl = sc_tile("warm")

        def warmup(n):
            for i in range(n):
                nc.tensor.matmul(wtl[:, 0, 0:128], lhsT=warm_src,
                                 rhs=warm_src, start=True, stop=True)

        warmup(W1)

        # remaining persistent tiles
        ident = P.tile([128, 128], f32, tag="ident")
        ident_bf = P.tile([128, 128], bf16, tag="ident_bf")
        et = [P.tile([128, 2, NQ], bf16, tag=f"et{s}", name=f"et{s}")
              for s in range(32)]
        onorm = P.tile([128, 4, D], bf16, tag="onorm")
        oT = P.tile([128, 2, NQ], bf16, tag="oT")
        out_sb = P.tile([128, 4, D], f32, tag="out_sb")
        h1_sb = P.tile([32, N], f32r, tag="h1_sb")
        ep = P.tile([H, N], f32, tag="ep")
        eps_bf = P.tile([H, N], bf16, tag="eps_bf")
        epT = P.tile([128, 8, H], bf16, tag="epT")
        gr = P.tile([H, 1], f32, tag="gr")
        rp = P.tile([H, 1], f32, tag="rp")
        spsum = P.tile([H, 1], f32, tag="spsum")
        u_bf = P.tile([128, 2, 1], bf16, tag="u_bf")
        bo_eff = P.tile([1, D], bf16, tag="bo_eff")
        recip = [P.tile([128, 2, 4], f32, tag=f"rc{g}", name=f"rc{g}")
                 for g in range(4)]

        # PV accumulators: per (qh) tile [128, ic, ht, 33], bank-padded;
        # the mc=1 allocation reuses the mc=0 bank after its epilogue.
        def pv_tile(qh, mc):
            return ps_pv.tile([128, 2, 4, DH + 1], f32, tag=f"pv{qh}",
                              padded_shape=[128, 2, 4, 64],
                              name=f"pv{qh}_{mc}")
        pv_cur = {}

        # ---- projections ----
        def proj_q(mc):
            t = sc_tile(f"qp{mc}")
            p = t[:].rearrange("p a b -> p (a b)")[:, 0:NQ]
            for kc in range(2):
                nc.tensor.matmul(p, lhsT=wq[:, kc, mc * 128:(mc + 1) * 128],
                                 rhs=xq[:, kc, :], start=(kc == 0),
                                 stop=(kc == 1))
            nc.scalar.copy(out=qT8[mc][:, 0, :], in_=p)

        def proj_k(mc, nn, split_first=False):
            t = sc_tile(f"kp{mc}{nn}")
            p = t[:].rearrange("p a b -> p (a b)")[:, 0:NQ]
            for kc in range(2):
                nc.tensor.matmul(
                    p, lhsT=wk[:, kc, mc * 128:(mc + 1) * 128],
                    rhs=xkv[:, kc, nn * NQ:(nn + 1) * NQ],
                    start=(kc == 0), stop=(kc == 1))
            dst = kT8[mc][:, 0, nn * NQ:(nn + 1) * NQ]
            if split_first:
                # kc=0 chunk first so scores(0) can start asap
                nc.vector.tensor_copy(out=dst[:, 0:128], in_=p[:, 0:128])
                nc.vector.tensor_copy(out=dst[:, 128:NQ], in_=p[:, 128:NQ])
            else:
                nc.vector.tensor_copy(out=dst, in_=p)

        def proj_k1():
            # both nn halves of the mc=1 key projection in one psum tile
            t = sc_tile("kp1")
            p = t[:].rearrange("p a b -> p (a b)")
            for nn in range(2):
                for kc in range(2):
                    nc.tensor.matmul(
                        p[:, nn * NQ:(nn + 1) * NQ],
                        lhsT=wk[:, kc, 128:256],
                        rhs=xkv[:, kc, nn * NQ:(nn + 1) * NQ],
                        start=(kc == 0), stop=(kc == 1))
            nc.vector.tensor_copy(out=kT8[1][:, 0, :], in_=p)

        def proj_v2(i):
            # value projection for jc pair (2i, 2i+1); one fused copy
            t = sc_tile(f"vp{i}")
            for jj in range(2):
                jc = 2 * i + jj
                for kc in range(2):
                    nc.tensor.matmul(
                        t[:, jj, 0:D],
                        lhsT=xkv[:, kc, jc * 128:(jc + 1) * 128],
                        rhs=wv[:, kc, :], start=(kc == 0), stop=(kc == 1))
            nc.vector.tensor_copy(
                out=v_aug[:, 2 * i:2 * i + 2, :, 0:DH],
                in_=t[:, :, 0:D].rearrange("p a (h d) -> p a h d", h=H))

        # ---- pos branch ----
        def pos_h1():
            # h1 = relu(Wp1^T posT + bp1), [32(16 used), 1024]; operands
            # zero-padded to K=M=32 (walrus ISA checks reject psum dst
            # partitions != 0 for f32r matmuls, so chunks stack along the
            # free dim instead of partitions).
            t = sc_tile("posh")
            v = t[:].rearrange("p a b -> p (a b)")[0:32, :]
            for c in range(2):
                nc.tensor.matmul(v[:, 512 * c:512 * (c + 1)], lhsT=wp1,
                                 rhs=posT[:, 512 * c:512 * (c + 1)],
                                 start=True, stop=True)
            nc.vector.tensor_scalar(out=h1_sb, in0=v, scalar1=bp1r,
                                    scalar2=0.0, op0=AL.add, op1=AL.max)

        def pos_sp():
            # sp[h, j] = w2h^T h1 (+cb); ep = exp(-sp - cb) with the cb
            # fold applied through the activation bias.
            t = sc_tile("possp")
            v = t[:].rearrange("p a b -> p (a b)")[0:H, :]
            for c in range(2):
                nc.tensor.matmul(
                    v[:, 512 * c:512 * (c + 1)],
                    lhsT=w2h,
                    rhs=h1_sb[:, 512 * c:512 * (c + 1)],
                    start=True, stop=True)
            nc.scalar.activation(out=ep, in_=v, func=EXP, scale=-1.0,
                                 bias=ncb, accum_out=spsum)
            nc.vector.reciprocal(out=rp, in_=spsum)
            nc.vector.tensor_tensor(out=gr, in0=gdm, in1=rp, op=AL.mult)
            nc.gpsimd.tensor_scalar(out=eps_bf, in0=ep, scalar1=gr,
                                    scalar2=0.0, op0=AL.mult, op1=AL.add)

        def pos_epT():
            # 8 transposes into one psum tile, then a single 64-free copy
            t = sc_tile("posep")
            pb = t[:].rearrange("p a b -> p (a b)").bitcast(bf16)
            for jc in range(8):
                nc.tensor.transpose(
                    pb[:, jc * 256:jc * 256 + H],
                    eps_bf[:, jc * 128:(jc + 1) * 128],
                    ident_bf[0:H, 0:H])
            nc.vector.tensor_copy(
                out=epT,
                in_=pb.rearrange("p (a c) -> p a c", c=256)[:, 0:8, 0:H])

        def pos_u():
            t = sc_tile("posu")
            p = t[:].rearrange("p a b -> p (a b)")
            for h in range(H):
                hp = (h % 4) * DH
                co = (h // 4) * 512          # separate 2KB bank per u column
                for jc in range(8):
                    nc.tensor.matmul(
                        p[hp:hp + DH, co:co + 1],
                        lhsT=v_aug[:, jc, h, 0:DH],
                        rhs=epT[:, jc, h:h + 1],
                        start=(jc == 0), stop=(jc == 7),
                        tile_position=(0, hp))
            nc.vector.tensor_copy(
                out=u_bf,
                in_=p.rearrange("p (a b) -> p a b", a=2)[:, :, 0:1])

        def pos_bo():
            t = sc_tile("posbo")
            p = t[:].rearrange("p a b -> p (a b)")[0:1, 0:D]
            for mc in range(2):
                nc.tensor.matmul(p, lhsT=u_bf[:, mc, :],
                                 rhs=wo_bf[:, mc, :], start=(mc == 0),
                                 stop=(mc == 1))
            nc.vector.tensor_tensor(out=bo_eff, in0=p, in1=bo_row, op=AL.add)

        # epilogue for pv group g=(mc,qh): normalize into onorm columns
        def epilogue(g):
            mc, qh = g // 2, g % 2
            pvt = pv_cur[(mc, qh)]
            nc.vector.reciprocal(out=recip[g], in_=pvt[:, :, :, DH:DH + 1])
            dst = onorm[:, qh * 2:qh * 2 + 2, mc * 128:(mc + 1) * 128] \
                .rearrange("p a (h d) -> p a h d", h=4)
            nc.vector.tensor_tensor(
                out=dst, in0=pvt[:, :, :, 0:DH],
                in1=recip[g][:, :, :].unsqueeze(3)
                    .broadcast_to([128, 2, 4, DH]),
                op=AL.mult)

        # transpose onorm[:, qt, mcd-half] -> oT[:, mcd, qt-slice]
        def out_tr(i, mcd):
            # qt pair (2i, 2i+1) for column-half mcd; the mid-loop (mcd=0)
            # copies go on DVE to keep ACT's exp stream (the loop pacer)
            # clear; the tail (mcd=1) copies alternate.
            t = sc_tile(f"tr{i}_{mcd}")
            pb = t[:].rearrange("p a b -> p (a b)").bitcast(bf16)
            for k in range(2):
                qt = 2 * i + k
                nc.tensor.transpose(
                    pb[:, k * 1024:k * 1024 + 128],
                    onorm[:, qt, mcd * 128:(mcd + 1) * 128],
                    ident_bf)
            for k in range(2):
                qt = 2 * i + k
                src = pb[:, k * 1024:k * 1024 + 128]
                dst = oT[:, mcd, qt * 128:(qt + 1) * 128]
                if mcd == 1 and qt % 2 == 0:
                    nc.scalar.copy(out=dst, in_=src)
                else:
                    nc.vector.tensor_copy(out=dst, in_=src)

        def out_proj(qt):
            t2 = sc_tile(f"op{qt}")
            po = t2[:].rearrange("p a b -> p (a b)")[:, 0:D]
            for mcd in range(2):
                nc.tensor.matmul(po, lhsT=oT[:, mcd, qt * 128:(qt + 1) * 128],
                                 rhs=wo_bf[:, mcd, :], start=(mcd == 0),
                                 stop=False)
            nc.tensor.matmul(po, lhsT=ones_bf, rhs=bo_eff, start=False,
                             stop=True)
            if qt % 2 == 0:
                nc.scalar.copy(out=out_sb[:, qt, :], in_=po)
            else:
                nc.vector.tensor_copy(out=out_sb[:, qt, :], in_=po)
            # ship output as soon as each piece is ready: the final DMA's
            # fixed latency chain (gen+delay+transfer+sem) gates the end
            if qt == 1:
                nc.sync.dma_start(out=dout_r[:, 0:2, :],
                                  in_=out_sb[:, 0:2, :])
            elif qt == 2:
                nc.scalar.dma_start(out=dout_r[:, 2:3, :],
                                    in_=out_sb[:, 2:3, :])
            elif qt == 3:
                nc.sync.dma_start(out=dout_r[:, 3:4, :],
                                  in_=out_sb[:, 3:4, :])

        # ---- main software-pipelined attention loop ----
        def scores(t, p, tl):
            mc = t // 8
            kc = t % 8
            for h2 in range(2):
                ht = 2 * p + h2
                base = ht * DH
                if ABL_FP8:
                    nc.tensor.matmul(
                        tl[:, h2, :],
                        lhsT=kT8[mc][base:base + DH, :,
                                     kc * 128:(kc + 1) * 128],
                        rhs=qT8[mc][base:base + DH, :, :],
                        start=True, stop=True, perf_mode=DR,
                        tile_position=(base, 0))
                else:
                    nc.tensor.matmul(
                        tl[:, h2, :],
                        lhsT=kT8[mc][base:base + DH, 0,
                                     kc * 128:(kc + 1) * 128],
                        rhs=qT8[mc][base:base + DH, 0, :],
                        start=True, stop=True,
                        tile_position=(base, 0))

        def exp_sub(t, p, tl):
            s = t * 2 + p
            if EXP_SPLIT:
                c = EXP_SPLIT
                nc.scalar.activation(out=et[s][:, :, 0:c],
                                     in_=tl[:, :, 0:c], func=EXP,
                                     scale=INV_C)
                nc.vector.tensor_scalar(
                    out=et[s][:, :, c:NQ].bitcast(i16),
                    in0=tl[:, :, c:NQ], scalar1=SA, scalar2=SB,
                    op0=AL.mult, op1=AL.add)
            elif t == 15:
                # split the final exps by query-half across both engines so
                # pv(15) for qh=0 can start before the full tile is done
                nc.scalar.activation(out=et[s][:, :, 0:256],
                                     in_=tl[:, :, 0:256], func=EXP,
                                     scale=INV_C)
                nc.vector.tensor_scalar(
                    out=et[s][:, :, 256:NQ].bitcast(i16),
                    in0=tl[:, :, 256:NQ], scalar1=SA, scalar2=SB,
                    op0=AL.mult, op1=AL.add)
            elif (t, p) in DVE_EXP:
                nc.vector.tensor_scalar(out=et[s][:].bitcast(i16), in0=tl,
                                        scalar1=SA, scalar2=SB,
                                        op0=AL.mult, op1=AL.add)
            else:
                nc.scalar.activation(out=et[s], in_=tl, func=EXP, scale=INV_C)

        def pv_mms(t, qhs=(0, 1)):
            mc = t // 8
            kc = t % 8
            if kc == 0:
                for qh in qhs:
                    pv_cur[(mc, qh)] = pv_tile(qh, mc)
            for qh in qhs:
                pvt = pv_cur[(mc, qh)]
                for p in range(2):
                    s = t * 2 + p
                    for h2 in range(2):
                        ht = 2 * p + h2
                        for icq in range(2):
                            ic = qh * 2 + icq
                            nc.tensor.matmul(
                                pvt[:, icq, ht, :],
                                lhsT=et[s][:, h2, ic * 128:(ic + 1) * 128],
                                rhs=v_aug[:, kc, mc * 4 + ht, :],
                                start=(kc == 0 and p == 0 and h2 == 0
                                       and icq == 0),
                                stop=(kc == 7 and p == 1 and h2 == 1
                                      and icq == 1))

        def make_ident():
            from concourse.masks import make_identity
            make_identity(nc, ident[:])
            nc.gpsimd.tensor_copy(out=ident_bf, in_=ident)

        # first q/k projections ahead of the loop (critical path);
        # dep-free warmup between them covers the xkv DMA wait.
        proj_q(0)
        warmup(W2)
        proj_k(0, 0, split_first=True)

        leftovers = {
            0: [lambda: proj_k(0, 1), lambda: proj_v2(0)],
            1: [lambda: proj_v2(1), make_ident],
            2: [lambda: proj_v2(2)],
            3: [lambda: proj_v2(3)],
            4: [lambda: proj_q(1)],
            5: [lambda: proj_k(1, 0)],
            6: [lambda: proj_k(1, 1)],
            7: [pos_h1],
            8: [pos_sp],
            9: [lambda: epilogue(0), lambda: epilogue(1)],
            10: [lambda: out_tr(0, 0)],
            11: [lambda: out_tr(1, 0)],
            12: [pos_epT],
            13: [pos_u],
            14: [pos_bo],
        }
        if LSCHED == 2:
            for t in (10, 11, 12, 13, 14):
                leftovers.pop(t)
            leftovers.update({
                10: [pos_epT], 11: [lambda: out_tr(0, 0)],
                12: [pos_u], 13: [lambda: out_tr(1, 0)],
                14: [pos_bo]})
        elif LSCHED == 3:
            for t in (10, 11, 12, 13, 14):
                leftovers.pop(t)
            leftovers.update({
                10: [lambda: out_tr(0, 0), pos_epT],
                11: [lambda: out_tr(1, 0), pos_u],
                12: [pos_bo]})

        for t in range(min(ABL_NT, 16)):
            for p in range(2):
                tl = sc_tile(f"s{t}_{p}")
                scores(t, p, tl)
                exp_sub(t, p, tl)
            for fn in leftovers.get(t, []):
                fn()
            if t > 0:
                pv_mms(t - 1)

        # tail: finish pv per qh-group; the qh0 output chain (transpose,
        # project, first DMA) overlaps the qh1 pv/epilogue work.
        if TAILV == 0:
            pv_mms(15, qhs=(0,))
            epilogue(2)
            out_tr(0, 1)
            pv_mms(15, qhs=(1,))
            epilogue(3)
            out_proj(0)
            out_proj(1)
            out_tr(1, 1)
            out_proj(2)
            out_proj(3)
        else:
            pv_mms(15, qhs=(0,))
            epilogue(2)
            pv_mms(15, qhs=(1,))
            epilogue(3)
            out_tr(0, 1)
            out_proj(0)
            out_proj(1)
            out_tr(1, 1)
            out_proj(2)
            out_proj(3)

    nc.compile()
    return nc


def _get_nc():
    if "nc" not in _nc_cache:
        _nc_cache["nc"] = _build_nc()
    return _nc_cache["nc"]


def _host_prep(inputs):
    x = np.ascontiguousarray(np.asarray(inputs["x"], dtype=np.float32))
    pos = np.ascontiguousarray(np.asarray(inputs["pos"], dtype=np.float32))
    Wq = np.asarray(inputs["Wq"], np.float32)
    Wk = np.asarray(inputs["Wk"], np.float32)
    Wv = np.asarray(inputs["Wv"], np.float32)
    Wo = np.asarray(inputs["Wo"], np.float32)
    bo = np.asarray(inputs["bo"], np.float32).reshape(1, D)
    Wp1 = np.asarray(inputs["Wp1"], np.float32)
    bp1 = np.asarray(inputs["bp1"], np.float32)
    Wp2 = np.asarray(inputs["Wp2"], np.float32)
    bp2 = np.asarray(inputs["bp2"], np.float32)
    Wh = np.asarray(inputs["Wh"], np.float32)
    bh = np.asarray(inputs["bh"], np.float32)
    gate = np.asarray(inputs["gate"], np.float32)
    g = (1.0 / (1.0 + np.exp(-gate.astype(np.float64))))
    omg = (1.0 - g)                          # (1-g), float64
    Wo_s = (Wo.astype(np.float64) *
            np.repeat(omg, DH)[:, None]).astype(np.float32)
    gdm = (g / omg).astype(np.float32).reshape(H, 1)

    w2h = (Wp2.astype(np.float64) @ Wh.astype(np.float64)).astype(np.float32)
    cb = (bp2.astype(np.float64) @ Wh.astype(np.float64)
          + bh.astype(np.float64)).astype(np.float32)

    import ml_dtypes

    def pack2(w):
        return np.ascontiguousarray(
            np.stack([w[0:128], w[128:256]], axis=1)
            .astype(ml_dtypes.bfloat16))

    blob = np.zeros((32, 640), np.float32)
    blob[0:PD, 0:PD] = Wp1      # zero-padded to [32, 32] for exact PE tiles
    blob[0:PD, 48:56] = w2h     # rows 16-31 stay zero
    blob[0:PD, 56] = bp1
    blob[0:H, 57:58] = gdm
    blob[0:H, 58] = -cb
    blob[0:1, 320:576] = bo
    per_core = []
    for core in range(NCORES):
        b, half = divmod(core, 2)
        q0 = half * NQ
        xT = np.ascontiguousarray(x[b].T)           # [256, 1024]
        posT32 = np.zeros((32, N), np.float32)
        posT32[0:PD] = pos[b].T
        per_core.append({
            "xq": pack2(np.ascontiguousarray(xT[:, q0:q0 + NQ])),
            "xkv": pack2(xT),
            "posT": posT32,
            "qkw": pack2(np.concatenate([Wq, Wk], axis=1)),
            "wvo": pack2(np.concatenate([Wv, Wo_s], axis=1)),
            "blob": blob,
        })
    return per_core


def kernel(**inputs):
    from concourse.bass_utils import run_bass_kernel_spmd

    nc = _get_nc()
    in_maps = _host_prep(inputs)
    res = run_bass_kernel_spmd(nc, in_maps, core_ids=list(range(NCORES)))
    out = np.empty((B, N, D), np.float32)
    for core in range(NCORES):
        b, half = divmod(core, 2)
        out[b, half * NQ:(half + 1) * NQ, :] = res.results[core]["out"]
    return out
